# revision 1
# baseline (speedup 1.0000x reference)
"""2x nearest-neighbor upsample of complex (real+imag) NHWC images on 8 trn2 cores.

out[t, b, i, j, c] = x_t[b, i // 2, j // 2, c]   (t = real/imag)

Strategy (data-parallel over batch, 2 images per core):
  - load half an input image row-block into SBUF ([128 rows, 64*64 f32] per half)
  - expand W in SBUF with DVE broadcast copies (each 64-float C-chunk doubled),
    materializing BOTH duplicated output rows in one [128, 16384] tile
  - one store per tile: 3-dim DRAM AP [row i: 128][copy r: 2][8192 contig]
    (walrus caps sync waits per instruction, so fewer DMAs per tile = fewer
    distinct DMA-lane semaphores any instruction must wait on)
HBM traffic per core = 16 MiB read + 64 MiB write (the minimum).
"""

import sys

import numpy as np

if "/opt/trn_rl_repo" not in sys.path:
    sys.path.insert(0, "/opt/trn_rl_repo")

import concourse.bass as bass
import concourse.bass_isa as bass_isa
import concourse.mybir as mybir
import concourse.tile_sem_assignment as _tsa
from concourse.bass_utils import run_bass_kernel_spmd
from concourse.tile import TileContext
from concourse.tile_rust import add_dep_helper

# Partition HWDGE DMA-completion semaphore lanes by issuing engine: SP
# (loads) on lane 0, ACT (stores) on lanes 2-7. Each lane then carries
# DMAs from a single HWDGE FIFO ring (per-lane completion order is
# trivially sound), and a DMA's own-lane predecessor is always one the
# issuing engine has already observed — keeping every DMA at the 1
# sync-wait walrus codegen allows.
_orig_assign_tick = _tsa.TileClockTick._assign_tick


def _assign_tick_lane_split(self, inst):
    if isinstance(inst, _tsa.DMAInst) and not isinstance(
        inst, bass_isa.UserSyncedRemoteDMADescs
    ):
        if inst.engine == mybir.EngineType.Pool:
            self.next_sw_dma_idx = 0
        elif inst.engine == mybir.EngineType.SP:
            self.next_hw_dma_idx = 0
        elif inst.engine == mybir.EngineType.Activation:
            r = getattr(self, "_act_lane_rr", 0)
            self.next_hw_dma_idx = 2 + r
            self._act_lane_rr = (r + 1) % 6
    return _orig_assign_tick(self, inst)


_tsa.TileClockTick._assign_tick = _assign_tick_lane_split

F32 = mybir.dt.float32

B, H, W, C = 16, 128, 128, 64
N_CORES = 8
BPC = B // N_CORES  # images per core

# W-chunk schedule per (tensor, image): halves in steady state; quarters
# for the first image (stores start sooner -> shorter pipeline ramp) and
# the last image (the final store drains sooner -> shorter tail).
_CHUNKS: list[list[tuple[int, int]]] = []
for _t in range(2):
    for _b in range(BPC):
        if _t == 0 and _b == 0:
            _CHUNKS.append([(0, W // 4), (W // 4, W // 4), (W // 2, W // 2)])
        elif _t == 1 and _b == BPC - 1:
            _CHUNKS.append([(0, W // 2), (W // 2, W // 4), (3 * W // 4, W // 4)])
        else:
            _CHUNKS.append([(0, W // 2), (W // 2, W // 2)])
N_ITERS = sum(len(c) for c in _CHUNKS)


def _build() -> bass.Bass:
    nc = bass.Bass("TRN2", debug=False)
    xr = nc.dram_tensor("x_real", [BPC, H, W, C], F32, kind="ExternalInput").ap()
    xi = nc.dram_tensor("x_imag", [BPC, H, W, C], F32, kind="ExternalInput").ap()
    out = nc.dram_tensor(
        "out", [2, BPC, 2 * H, 2 * W, C], F32, kind="ExternalOutput"
    ).ap()
    WH = W // 2  # input W columns per half-tile

    HALF = 2 * WH * C  # expanded half-row length (8192 f32 = 32 KB)

    # walrus codegen allows exactly ONE sync-wait command per engine
    # instruction (multi-wait is only legal on Drain/EventSemaphore).
    # Tile emits a wait only when the issuing engine has not already
    # observed that semaphore tick through an earlier *real* instruction's
    # wait (NoOps don't count). So every instruction below is budgeted to
    # observe at most one fresh tick, using tiny "absorber" instructions
    # (1-element memsets on DVE, 2-element probe copies on ACT, 4-byte
    # writes on SP) to pre-observe everything else.
    #
    # Loads issue from the SP HWDGE ring and stores from the ACT ring so
    # load prefetch is never blocked behind a store's data wait; each
    # store half fires as soon as its own DVE copy finishes.
    with TileContext(nc) as tc:
        with (
            tc.tile_pool(name="pin", bufs=2) as pin,
            tc.tile_pool(name="pinit", bufs=2) as pinit,
            tc.tile_pool(name="pout", bufs=2) as pout,
            tc.tile_pool(name="pdummy", bufs=1) as pdummy,
        ):
            dummy = pdummy.tile([H, 4 * N_ITERS], F32, name="dummy")
            vdummy = pdummy.tile([H, 3 * N_ITERS], F32, name="vdummy")
            pooldummy = pdummy.tile([1, N_ITERS], F32, name="pooldummy")
            spdummy = pdummy.tile([1, 16], F32, name="spdummy")
            last_pabs = None
            st_los = []
            st_his = []
            aabs1s = []
            dmas = []
            cps_all = []
            k = 0
            for t, x in enumerate((xr, xi)):
                for b in range(BPC):
                    # partition i holds input row i, feeding output rows 2i, 2i+1
                    ov = out[t, b].rearrange("(i r) w c -> i r (w c)", r=2)
                    for w0, wlen in _CHUNKS[t * BPC + b]:
                        EXP = 2 * wlen * C  # expanded chunk per output row copy
                        if k < 2:
                            # dedicated, never-recycled tiles: the first two
                            # loads carry no WAW/WAR deps at all
                            tin = pinit.tile(
                                [H, (W // 4) * C], F32, name="tin_init"
                            )
                        else:
                            tin = pin.tile([H, WH * C], F32, name="tin")
                        # Pool-side absorber (gpsimd memset = a real engine
                        # instruction): observe DVE at the newest finished
                        # copy so the load's WAR on its recycled tin slot
                        # (and the slot-release bundle, which lands later
                        # on the DVE timeline than the slot's accessors)
                        # needs no fresh DVE wait.
                        if k >= 2:
                            pabs = nc.gpsimd.memset(pooldummy[:1, k : k + 1], 0.0)
                            add_dep_helper(
                                pabs.ins, cps_all[-1].ins, sync=True,
                                reason="Pool observes DVE for load WAR",
                            )
                            last_pabs = pabs
                            ld = nc.gpsimd.dma_start(
                                out=tin[:, : wlen * C],
                                in_=x[b, :, w0 : w0 + wlen, :],
                            )
                            add_dep_helper(
                                ld.ins, pabs.ins, sync=False,
                                reason="absorber runs before load",
                            )
                        else:
                            # first two chunks: fresh slots, no WAR -> use
                            # the otherwise-idle SP HWDGE ring (faster
                            # first byte than the SWDGE Q7 path)
                            ld = nc.sync.dma_start(
                                out=tin[:, : wlen * C],
                                in_=x[b, :, w0 : w0 + wlen, :],
                            )
                        tout = pout.tile([H, 2 * HALF], F32, name="tout")
                        # DVE-side absorbers: per-iter distinct scratch
                        # cells (no WAW chains), not touching tout (the
                        # slot-release bundle must land on cp0, after the
                        # absorbers already observed all of it).
                        vabs1 = nc.vector.memset(vdummy[:1, 3 * k : 3 * k + 1], 0.0)
                        vabs2 = nc.vector.memset(
                            vdummy[:1, 3 * k + 1 : 3 * k + 2], 0.0
                        )
                        vabs3 = nc.vector.memset(
                            vdummy[:1, 3 * k + 2 : 3 * k + 3], 0.0
                        )
                        if k >= 2:
                            add_dep_helper(
                                vabs1.ins, st_los[k - 2].ins, sync=True,
                                reason="absorb tout slot WAR (store-lo lane)",
                            )
                            add_dep_helper(
                                vabs2.ins, st_his[k - 2].ins, sync=True,
                                reason="absorb tout slot WAR (store-hi lane)",
                            )
                        if k >= 1:
                            add_dep_helper(
                                vabs3.ins, aabs1s[k - 1].ins, sync=True,
                                reason="absorb probe WAR (ACT sem)",
                            )
                        src = (
                            tin[:, : wlen * C]
                            .rearrange("p (w c) -> p w c", c=C)
                            .unsqueeze(2)
                            .broadcast_to([H, wlen, 2, C])
                        )
                        cps = []
                        for r in range(2):
                            dst = tout[:, r * EXP : (r + 1) * EXP].rearrange(
                                "p (w s c) -> p w s c", s=2, c=C
                            )
                            cp = nc.vector.tensor_copy(out=dst, in_=src)
                            for vb in (vabs1, vabs2, vabs3):
                                add_dep_helper(
                                    cp.ins, vb.ins, sync=False,
                                    reason="absorbers run before copies",
                                )
                            cps.append(cp)
                        add_dep_helper(
                            cps[1].ins, cps[0].ins, sync=True,
                            reason="DVE self-sem watermark",
                        )
                        # Each store half fires right after its own copy;
                        # a 2-element ACT probe of that copy's region
                        # absorbs the DVE data wait first.
                        o0 = 2 * w0 * C
                        aabs0 = nc.scalar.copy(
                            out=dummy[:1, 4 * k : 4 * k + 2],
                            in_=tout[:1, 0:2],
                        )
                        st_lo = nc.scalar.dma_start(
                            out=ov[:, 0, o0 : o0 + EXP],
                            in_=tout[:, :EXP],
                        )
                        add_dep_helper(
                            st_lo.ins, aabs0.ins, sync=False,
                            reason="probe runs before store",
                        )
                        aabs1 = nc.scalar.copy(
                            out=dummy[:1, 4 * k + 2 : 4 * k + 4],
                            in_=tout[:1, EXP : EXP + 2],
                        )
                        st_hi = nc.scalar.dma_start(
                            out=ov[:, 1, o0 : o0 + EXP],
                            in_=tout[:, EXP : 2 * EXP],
                        )
                        add_dep_helper(
                            st_hi.ins, aabs1.ins, sync=False,
                            reason="probe runs before store",
                        )
                        st_los.append(st_lo)
                        st_his.append(st_hi)
                        aabs1s.append(aabs1)
                        dmas.extend([ld, st_lo, st_hi])
                        cps_all.extend(cps)
                        k += 1
            # Kernel-tail absorbers: Tile's final SP drain waits on every
            # outstanding proc (DVE + ACT + 8 DMA lanes = 10 waits), but a
            # multi-wait drain lowers to a 1-wait NOP struct when cheap.
            # Pre-observe each proc with one 4-byte SP write per tick.
            # dmas[3] = the second SP-issued head load: its wait covers the
            # DMAHW0 lane both head loads completed on
            tail_deps = dmas[-8:] + [aabs1s[-1], cps_all[-1], last_pabs, dmas[3]]
            for j, dep in enumerate(tail_deps):
                wr = nc.sync.write(spdummy[:1, j : j + 1], b"\x00\x00\x00\x00")
                add_dep_helper(
                    wr.ins, dep.ins, sync=True,
                    reason="pre-observe outstanding procs for tail drain",
                )
    return nc


_NC_CACHE: bass.Bass | None = None


def _get_nc() -> bass.Bass:
    global _NC_CACHE
    if _NC_CACHE is None:
        _NC_CACHE = _build()
    return _NC_CACHE


def _run(x_real: np.ndarray, x_imag: np.ndarray, **spmd_kwargs):
    x_real = np.ascontiguousarray(np.asarray(x_real, dtype=np.float32))
    x_imag = np.ascontiguousarray(np.asarray(x_imag, dtype=np.float32))
    assert x_real.shape == (B, H, W, C), x_real.shape
    assert x_imag.shape == (B, H, W, C), x_imag.shape
    in_maps = [
        {
            "x_real": x_real[c * BPC : (c + 1) * BPC],
            "x_imag": x_imag[c * BPC : (c + 1) * BPC],
        }
        for c in range(N_CORES)
    ]
    res = run_bass_kernel_spmd(
        _get_nc(), in_maps, core_ids=list(range(N_CORES)), **spmd_kwargs
    )
    full = np.concatenate([r["out"] for r in res.results], axis=1)
    return full, res


def kernel(x_real: np.ndarray, x_imag: np.ndarray) -> np.ndarray:
    full, _ = _run(x_real, x_imag)
    return full



# revision 5
# speedup vs baseline: 1.1327x; 1.1327x over previous
"""2x nearest-neighbor upsample of complex (real+imag) NHWC images on 8 trn2 cores.

out[t, b, i, j, c] = x_t[b, i // 2, j // 2, c]   (t = real/imag)

Strategy (data-parallel over batch, 2 images per core):
  - prefetch the ENTIRE 16 MiB per-core input into 8 dedicated SBUF tiles
    (one per half-image chunk) via back-to-back HWDGE loads on the SP ring
    at t=0 — loads carry zero dependencies, so they are never on the
    chunk-to-chunk critical path (the old kernel serialized
    copy(k) -> load(k+1) -> copy(k+1) and idled the SDMA engines ~23%)
  - per chunk: ONE DVE broadcast copy expands W 2x into a [128, 8192] tile
    (partition i = input row i), then TWO 4 MiB HWDGE stores on the ACT
    ring write duplicated output rows 2i and 2i+1 from the SAME region
  - no SWDGE traffic at all (the SWDGE descriptor rings contend for the
    AXI ports of SDMA engines 7/15, which made engine 15 a straggler)
HBM traffic per core = 16 MiB read + 64 MiB write (the minimum); the
schedule keeps all 16 SDMA engines busy end-to-end.
"""

import sys

import numpy as np

if "/opt/trn_rl_repo" not in sys.path:
    sys.path.insert(0, "/opt/trn_rl_repo")

import concourse.bass as bass
import concourse.bass_isa as bass_isa
import concourse.mybir as mybir
import concourse.tile_sem_assignment as _tsa
from concourse.bass_utils import run_bass_kernel_spmd
from concourse.tile import TileContext
from concourse.tile_rust import add_dep_helper

# Partition HWDGE DMA-completion semaphore lanes by issuing engine: SP
# (loads) on lane 0, ACT (stores) on lanes 2-7. Each lane then carries
# DMAs from a single HWDGE FIFO ring (per-lane completion order is
# trivially sound), and a DMA's own-lane predecessor is always one the
# issuing engine has already observed — keeping every DMA at the 1
# sync-wait walrus codegen allows.
_orig_assign_tick = _tsa.TileClockTick._assign_tick


def _assign_tick_lane_split(self, inst):
    if isinstance(inst, _tsa.DMAInst) and not isinstance(
        inst, bass_isa.UserSyncedRemoteDMADescs
    ):
        if inst.engine == mybir.EngineType.Pool:
            self.next_sw_dma_idx = 0
        elif inst.engine == mybir.EngineType.SP:
            self.next_hw_dma_idx = 0
        elif inst.engine == mybir.EngineType.Activation:
            r = getattr(self, "_act_lane_rr", 0)
            self.next_hw_dma_idx = 2 + r
            self._act_lane_rr = (r + 1) % 6
    return _orig_assign_tick(self, inst)


_tsa.TileClockTick._assign_tick = _assign_tick_lane_split

F32 = mybir.dt.float32

B, H, W, C = 16, 128, 128, 64
N_CORES = 8
BPC = B // N_CORES  # images per core

WH = W // 2  # input W columns per chunk (half an image row)
IN_LEN = WH * C  # 4096 f32 = 16 KB per partition
EXP = 2 * IN_LEN  # expanded chunk (W doubled) = 8192 f32 = 32 KB
N_CHUNKS = 2 * BPC * 2  # (tensor, image, half) = 8


def _build() -> bass.Bass:
    nc = bass.Bass("TRN2", debug=False)
    xr = nc.dram_tensor("x_real", [BPC, H, W, C], F32, kind="ExternalInput").ap()
    xi = nc.dram_tensor("x_imag", [BPC, H, W, C], F32, kind="ExternalInput").ap()
    out = nc.dram_tensor(
        "out", [2, BPC, 2 * H, 2 * W, C], F32, kind="ExternalOutput"
    ).ap()

    # walrus codegen allows exactly ONE sync-wait command per engine
    # instruction. Tile emits a wait only when the issuing engine has not
    # already observed that semaphore tick through an earlier *real*
    # instruction's wait. Every instruction below is budgeted to observe
    # at most one fresh tick, using tiny "absorber" instructions
    # (1-element memsets on DVE, 2-element probe copies on ACT, 4-byte
    # writes on SP) to pre-observe everything else.
    with TileContext(nc) as tc:
        with (
            tc.tile_pool(name="pin", bufs=N_CHUNKS) as pin,
            tc.tile_pool(name="pout", bufs=2) as pout,
            tc.tile_pool(name="pdummy", bufs=1) as pdummy,
        ):
            adummy = pdummy.tile([1, 2 * N_CHUNKS], F32, name="adummy")
            vdummy = pdummy.tile([1, 4 * N_CHUNKS], F32, name="vdummy")
            spdummy = pdummy.tile([1, 16], F32, name="spdummy")

            chunks = [
                (x, b, h)
                for x in (xr, xi)
                for b in range(BPC)
                for h in range(2)
            ]

            # All loads first: dedicated tiles, no WAR/WAW deps -> the SP
            # engine fires all 8 back-to-back at t=0 on the SP HWDGE ring.
            tins = []
            loads = []
            for x, b, h in chunks:
                tin = pin.tile([H, IN_LEN], F32, name="tin")
                ld = nc.sync.dma_start(
                    out=tin[:, :], in_=x[b, :, h * WH : (h + 1) * WH, :]
                )
                tins.append(tin)
                loads.append(ld)

            probes = []
            st_los = []
            st_his = []
            copies = []
            for j, (x, b, h) in enumerate(chunks):
                t = 0 if x is xr else 1
                # partition i holds input row i, feeding output rows 2i, 2i+1
                ov = out[t, b].rearrange("(i r) w c -> i r (w c)", r=2)
                tout = pout.tile([H, EXP], F32, name="tout")
                # DVE-side absorbers: pre-observe the chunk j-2 slot
                # readers (its two stores on DMA lanes + its ACT probe)
                # so the copy's only fresh wait is load j's lane-0 tick.
                if j >= 2:
                    # DVE self-sem watermark: observing the previous
                    # copy's own-engine tick (trivially satisfied by
                    # program order) pre-observes the chunk j-2 tout
                    # slot-release tick, so the copy needs no self-wait.
                    vabs0 = nc.vector.memset(vdummy[:1, 4 * j : 4 * j + 1], 0.0)
                    add_dep_helper(
                        vabs0.ins, copies[j - 1].ins, sync=True,
                        reason="DVE self-sem watermark (slot release)",
                    )
                    vabs1 = nc.vector.memset(
                        vdummy[:1, 4 * j + 1 : 4 * j + 2], 0.0
                    )
                    add_dep_helper(
                        vabs1.ins, st_los[j - 2].ins, sync=True,
                        reason="absorb tout slot WAR (store-lo lane)",
                    )
                    vabs2 = nc.vector.memset(
                        vdummy[:1, 4 * j + 2 : 4 * j + 3], 0.0
                    )
                    add_dep_helper(
                        vabs2.ins, st_his[j - 2].ins, sync=True,
                        reason="absorb tout slot WAR (store-hi lane)",
                    )
                    vabs3 = nc.vector.memset(
                        vdummy[:1, 4 * j + 3 : 4 * j + 4], 0.0
                    )
                    add_dep_helper(
                        vabs3.ins, probes[j - 2].ins, sync=True,
                        reason="absorb probe WAR (ACT sem)",
                    )
                    vabss = (vabs0, vabs1, vabs2, vabs3)
                else:
                    vabss = ()
                src = (
                    tins[j]
                    .rearrange("p (w c) -> p w c", c=C)
                    .unsqueeze(2)
                    .broadcast_to([H, WH, 2, C])
                )
                dst = tout.rearrange("p (w s c) -> p w s c", s=2, c=C)
                cp = nc.vector.tensor_copy(out=dst, in_=src)
                for vb in vabss:
                    add_dep_helper(
                        cp.ins, vb.ins, sync=False,
                        reason="absorbers run before copy",
                    )
                copies.append(cp)
                # 2-element ACT probe of the copy's region absorbs the
                # DVE data wait; both stores then fire with only their
                # own-lane-predecessor wait.
                probe = nc.scalar.copy(
                    out=adummy[:1, 2 * j : 2 * j + 2], in_=tout[:1, 0:2]
                )
                probes.append(probe)
                o0 = h * EXP
                st_lo = nc.scalar.dma_start(
                    out=ov[:, 0, o0 : o0 + EXP], in_=tout[:, :]
                )
                add_dep_helper(
                    st_lo.ins, probe.ins, sync=False,
                    reason="probe runs before store",
                )
                st_hi = nc.scalar.dma_start(
                    out=ov[:, 1, o0 : o0 + EXP], in_=tout[:, :]
                )
                add_dep_helper(
                    st_hi.ins, probe.ins, sync=False,
                    reason="probe runs before store",
                )
                st_los.append(st_lo)
                st_his.append(st_hi)

            # Kernel-tail absorbers: Tile's final SP drain waits on every
            # outstanding proc, but a multi-wait drain lowers to a 1-wait
            # NOP struct when cheap. Pre-observe each proc with one
            # 4-byte SP write per tick: lane 0 via the last load, lanes
            # 2-7 via the last six stores, ACT via the last probe, DVE
            # via the last copy.
            tail_deps = [
                loads[-1],
                st_los[-3], st_his[-3],
                copies[-1], probes[-1],
                st_los[-2], st_his[-2],
                st_los[-1], st_his[-1],
            ]
            for j, dep in enumerate(tail_deps):
                wr = nc.sync.write(spdummy[:1, j : j + 1], b"\x00\x00\x00\x00")
                add_dep_helper(
                    wr.ins, dep.ins, sync=True,
                    reason="pre-observe outstanding procs for tail drain",
                )
    return nc


_NC_CACHE: bass.Bass | None = None


def _get_nc() -> bass.Bass:
    global _NC_CACHE
    if _NC_CACHE is None:
        _NC_CACHE = _build()
    return _NC_CACHE


def _run(x_real: np.ndarray, x_imag: np.ndarray, **spmd_kwargs):
    x_real = np.ascontiguousarray(np.asarray(x_real, dtype=np.float32))
    x_imag = np.ascontiguousarray(np.asarray(x_imag, dtype=np.float32))
    assert x_real.shape == (B, H, W, C), x_real.shape
    assert x_imag.shape == (B, H, W, C), x_imag.shape
    in_maps = [
        {
            "x_real": x_real[c * BPC : (c + 1) * BPC],
            "x_imag": x_imag[c * BPC : (c + 1) * BPC],
        }
        for c in range(N_CORES)
    ]
    res = run_bass_kernel_spmd(
        _get_nc(), in_maps, core_ids=list(range(N_CORES)), **spmd_kwargs
    )
    full = np.concatenate([r["out"] for r in res.results], axis=1)
    return full, res


def kernel(x_real: np.ndarray, x_imag: np.ndarray) -> np.ndarray:
    full, _ = _run(x_real, x_imag)
    return full
